# revision 60
# baseline (speedup 1.0000x reference)
"""LIF spike scan kernel for Trainium2 (8 NeuronCores, data-parallel).

Reference computation (per element, scanned over t):
    mem = mem * 0.2 * (1 - spk) + x[t]
    spk = (mem > 0.5)

Carry formulation with v = post-reset membrane (v = mem * (mem <= 0.5)):
    m = v * 0.2 + x[t]    -> DVE scalar_tensor_tensor, three ops aligned with
                             the v-producer ranges below
    b = (m < 0.5)         -> ACT Sign(0.5-m) -> saturating u8, exact {0,1};
                             three ops split at A1/A so each consumer waits
                             only its own range
    reset, columns [0, A):  v = b * m on Pool tensor_tensor mult (u8 x f32,
                             exact), two ops split at A1
    reset, columns [A, F):  v = (m <= 0.5) * m, DVE fused
                             scalar_tensor_tensor is_le+mult

Output is fully bit-packed on the PE with the u8 mask BITCAST to fp8e4:
byte 0x01 reads as the e4m3 subnormal 2^-9 (a power of two), so with weight
block j (j = t mod 8) holding W[8c+k, 16j+c] = 2^k in fp8e4, PSUM row 16j+c
accumulates 2^-9 * sum_k 2^k * b[8c+k] exactly over 8 timesteps -- all 128
rows useful. One ACT copy PSUM->u8 with scale=-512, bias=255 per group
recovers the true packed spikes (255 - sum 2^k (1-s) = sum 2^k s) exactly
(powers of two throughout), then one [128, 2048] u8 DMA per 8 timesteps.
Output traffic: 2.1 MB/core vs 67.1 MB input (baseline wrote 10.5 MB).
The bitcast removes the bf16 mask copy and the DVE is_le op entirely: one
sign per column serves the Pool reset AND the PE pack.

Weights are shipped from the host as raw fp8e4m3 bit patterns in a uint8
tensor (2^k = (k+7)<<3) and bitcast on device -- 128 KB instead of 512 KB
on the critical DMA stream and no conversion op.

Engine busy per tile-iteration (F=2048 cols, 64 iterations/core):
DMA ~3010 (in 2913 + out/weights), Pool ~2980, DVE ~2980, ACT ~2500,
PE ~850 -> DMA-bound at the 360 GB/s HBM input stream. Pool min/max and
gpsimd stt do not lower on trn2 HW (ISA check), hence the mult-based reset.
Group flips are emitted after both tiles' mask ops, the second pass's
t=0..2 fronts are software-pipelined under the first pass's tail (their
matmuls wait on the PSUM flips at the real boundary), the final timestep
splits its m-update three ways, and the final pass's first-tile flip runs
on the drain-idle DVE (tensor_scalar mult/add from PSUM) so the last
tile's sign chain never queues behind a 1.9us ACT flip.

Only m == 0.5 exactly deviates from the reference (spike recorded as 1 and
v reset to 0 instead of holding; measure-zero under random normal inputs --
4 of 134M elements on the seed-0 input). Everything else is bit-exact fp32.

Sharding: x is [T=16, B=64, C=128, H=32, W=32]; each core takes a contiguous
1/8 of the flattened B*C*H*W axis viewed as [T, 128, 8192]. Two q-tile
chains run concurrently per pass (PSUM holds two [128, 2048] f32 group
accumulators); two passes cover the four q-tiles.
"""

import numpy as np

T = 16
SPATIAL = 64 * 128 * 32 * 32  # 8388608
N_CORES = 8
NPC = SPATIAL // N_CORES      # 1048576 elements per core per timestep
P = 128                       # SBUF partitions
Q = NPC // P                  # 8192 free-dim columns per core
F = 2048                      # free-dim tile size
A = 1408                      # ACT-mask + Pool-mult columns; rest DVE vdn
A1 = 704                      # Pool mult / sign / stt split inside [0, A)
TG = 8                        # timesteps per packed PSUM group
NG = T // TG                  # groups per tile
DECAY = 0.2
THRESH = 0.5

_cache = {}

# Set by test harness to request an NTFF trace / HW timing.
TRACE = False


def _pack_weights() -> np.ndarray:
    """[128, 1024] uint8 holding fp8e4m3 bit patterns, 8 blocks of
    [128, 128]: block j has W[8c+b, 128j + 16j + c] = 2^b for c in [0,16),
    b in [0,8); else 0. fp8e4m3 2^b = (b+7)<<3. Block j lands timestep j's
    packed bytes at PSUM rows 16j..16j+15."""
    w = np.zeros((P, TG * P), dtype=np.uint8)
    for j in range(TG):
        for c in range(16):
            for b in range(8):
                w[8 * c + b, P * j + 16 * j + c] = (b + 7) << 3
    return w


def _build():
    from contextlib import ExitStack

    import concourse.bacc as bacc
    import concourse.tile as tile
    from concourse import mybir

    f32 = mybir.dt.float32
    fp8 = mybir.dt.float8e4
    u8 = mybir.dt.uint8
    Alu = mybir.AluOpType
    Act = mybir.ActivationFunctionType

    nc = bacc.Bacc("TRN2", target_bir_lowering=False, debug=False)
    x_d = nc.dram_tensor("x", [T, P, Q], f32, kind="ExternalInput").ap()
    w_d = nc.dram_tensor("w", [P, TG * P], u8, kind="ExternalInput").ap()
    # Packed planes: [q-tile, group, 128, F]; row 16j+c of group g holds
    # sum_b 2^b * spk[8g+j, 8c+b, col].
    p_d = nc.dram_tensor("pck", [4, NG, P, F], u8, kind="ExternalOutput").ap()

    # Const APs for activation biases (written pre-tile-region + barrier so
    # bias reads stay untracked).
    for name, val in (("thr", THRESH), ("zero", 0.0), ("flip", 255.0)):
        ap = nc.alloc_sbuf_tensor(f"const-f32-{name}", [128, 1], f32)
        nc.gpsimd.memset(ap.ap(), val)
        nc.const_aps.aps[(f32, val)] = ap.ap()
    nc.all_engine_barrier()

    with tile.TileContext(nc) as tc, ExitStack() as ctx:
        wpool = ctx.enter_context(tc.tile_pool(name="wgt", bufs=1))
        xpool = ctx.enter_context(tc.tile_pool(name="xin", bufs=11))
        vpool = ctx.enter_context(tc.tile_pool(name="vst", bufs=10))
        bpool = ctx.enter_context(tc.tile_pool(name="bms", bufs=9))
        opool = ctx.enter_context(tc.tile_pool(name="out", bufs=4))
        ppool = ctx.enter_context(tc.tile_pool(name="acc", bufs=2, space="PSUM"))

        wu8 = wpool.tile([P, TG * P], u8)
        wb = wu8.bitcast(fp8)

        pend = []  # deferred flip+store: (ti, group, psum tile)

        def flush(n=None, tail=False):
            todo = pend[:n] if n else pend[:]
            del pend[: len(todo)]
            for i, (ti, g, pacc) in enumerate(todo):
                o = opool.tile([P, F], u8, name="po")
                if tail and i == 0:
                    # final pass, first tile: flip on the (idle-at-drain) DVE
                    # so the last tile's sign ops never queue behind a 1.9us
                    # ACT flip
                    nc.vector.tensor_scalar(
                        o[:], pacc[:], -512.0, 255.0,
                        op0=Alu.mult, op1=Alu.add,
                    )
                else:
                    nc.scalar.activation(
                        o[:], pacc[:], Act.Copy, scale=-512.0, bias=255.0
                    )
                nc.scalar.dma_start(p_d[ti, g, :, :], o[:])

        first = True
        state = {}

        def load(ti, t):
            q0 = ti * F
            xt = xpool.tile([P, F], f32, name="xt")
            nc.sync.dma_start(xt[:], x_d[t, :, q0 : q0 + F])
            return xt

        def front_tail(ti, t, vprev):
            """Final tile of the run: chunked load so mask compute pipelines
            under the transfer -- only ~1.4us of work remains after the last
            512-col chunk lands. t=T-1 needs only the pack mask (no reset);
            Sign on the D-range too only moves the measure-zero m==0.5 case."""
            q0 = ti * F
            xt = xpool.tile([P, F], f32, name="xt")
            b = bpool.tile([P, F], u8, name="b")
            for c in range(F // 512):
                ch = slice(512 * c, 512 * (c + 1))
                nc.sync.dma_start(xt[:, ch], x_d[t, :, q0 + 512 * c : q0 + 512 * (c + 1)])
                if vprev is not None:
                    nc.vector.scalar_tensor_tensor(
                        xt[:, ch], vprev[:, ch], DECAY, xt[:, ch],
                        op0=Alu.mult, op1=Alu.add,
                    )
                nc.scalar.activation(
                    b[:, ch], xt[:, ch], Act.Sign, scale=-1.0, bias=THRESH
                )
            return b, None

        def front(ti, t, vprev, xt=None):
            """x load + m-update + masks + reset for (ti, t); mats deferred."""
            nonlocal first
            if xt is None:
                xt = load(ti, t)
            if first:
                # weight bytes (fp8e4 bit patterns) queued behind the first
                # x tile so x[0] isn't delayed
                nc.sync.dma_start(wu8[:], w_d)
                first = False
            m = xt
            b = bpool.tile([P, F], u8, name="b")
            vn = vpool.tile([P, F], f32, name="vn") if t < T - 1 else None
            # m-update in three ops aligned with the apply ranges so each
            # only waits its own v producer
            if vprev is not None:
                # 3-way split at the last step shortens the drain chain
                # (sign3 waits only the short [A,F) piece)
                rs = ((0, A1), (A1, A), (A, F)) if t == T - 1 else ((0, A1), (A1, F))
                for r0, r1 in rs:
                    nc.vector.scalar_tensor_tensor(
                        m[:, r0:r1], vprev[:, r0:r1], DECAY,
                        m[:, r0:r1], op0=Alu.mult, op1=Alu.add,
                    )
            # anti-spike mask b = (m < 0.5): ACT Sign(0.5-m) -> saturating u8,
            # exact {0,1}; split at A1/A so each consumer waits only its range
            nc.scalar.activation(
                b[:, 0:A1], m[:, 0:A1], Act.Sign, scale=-1.0, bias=THRESH
            )
            nc.scalar.activation(
                b[:, A1:A], m[:, A1:A], Act.Sign, scale=-1.0, bias=THRESH
            )
            nc.scalar.activation(
                b[:, A:F], m[:, A:F], Act.Sign, scale=-1.0, bias=THRESH
            )
            if vn is not None:
                # hard reset v = mask * m. The u8 b feeds Pool directly
                # (exact {0,1} multiply).
                nc.gpsimd.tensor_tensor(
                    vn[:, 0:A1], b[:, 0:A1], m[:, 0:A1], op=Alu.mult
                )
                nc.gpsimd.tensor_tensor(
                    vn[:, A1:A], b[:, A1:A], m[:, A1:A], op=Alu.mult
                )
                # [A,F): fused compare+multiply on DVE
                nc.vector.scalar_tensor_tensor(
                    vn[:, A:F], m[:, A:F], THRESH, m[:, A:F],
                    op0=Alu.is_le, op1=Alu.mult,
                )
            return b, vn

        def mats(ti, t, b, acc):
            """pack matmuls for (ti, t) into acc. The u8 mask is bitcast to
            fp8e4 (0x01 == 2^-9 exactly), so PSUM accumulates
            2^-9 * sum_k 2^k * b -- recovered exactly by the flip copy's
            scale=-512."""
            j = t % TG
            for c in range(F // 512):
                ch = slice(512 * c, 512 * (c + 1))
                nc.tensor.matmul(
                    acc[:, ch], wb[:, P * j : P * (j + 1)],
                    b[:, ch].bitcast(mybir.dt.float8e4),
                    start=(j == 0), stop=(j == TG - 1),
                    skip_group_check=True,
                )
            return acc

        for pair in range(2):
            tiles = (2 * pair, 2 * pair + 1)
            v = {ti: None for ti in tiles}
            acc = {ti: None for ti in tiles}
            for t in range(T):
                j = t % TG
                am = {}
                for ti in tiles:
                    if (pair, ti, t) in state:
                        a, vn = state.pop((pair, ti, t))
                    elif pair == 1 and ti == tiles[1] and t == T - 1:
                        a, vn = front_tail(ti, t, v[ti])
                    else:
                        a, vn = front(ti, t, v[ti])
                    v[ti] = vn
                    am[ti] = a
                if j == 0:
                    # flip+store the previous groups after BOTH tiles' mask
                    # ops (no ACT head-of-line burst at the boundary), then
                    # reallocate the PSUM accumulators (matmuls below wait
                    # on the flips via the pool slots)
                    for ti in tiles:
                        if acc[ti] is not None:
                            pend.append((ti, t // TG - 1, acc[ti]))
                    flush()
                    for ti in tiles:
                        acc[ti] = ppool.tile([P, F], f32, name="acc")
                for ti in tiles:
                    mats(ti, t, am[ti], acc[ti])
                if pair == 0 and t >= T - 3:
                    # software-pipeline the pass boundary: the next pass's
                    # first fronts (t=0 has no recurrence inputs; t=1,2 chain
                    # off the prefetched resets) run under this pass's tail;
                    # only their matmuls wait (on the PSUM flips) at the real
                    # boundary
                    nt = t - (T - 3)
                    for nti in (tiles[0] + 2, tiles[1] + 2):
                        vprev = state[(1, nti, nt - 1)][1] if nt else None
                        state[(1, nti, nt)] = front(nti, nt, vprev)
            for ti in tiles:
                pend.append((ti, (T - 1) // TG, acc[ti]))
            flush(tail=(pair == 1))
    nc.compile()
    return nc


def kernel(x: np.ndarray) -> np.ndarray:
    from concourse.bass_utils import run_bass_kernel_spmd

    if "nc" not in _cache:
        _cache["nc"] = _build()
    nc = _cache["nc"]

    x = np.ascontiguousarray(x, dtype=np.float32).reshape(T, N_CORES, NPC)
    w = _pack_weights()
    in_maps = [
        {"x": np.ascontiguousarray(x[:, i]).reshape(T, P, Q), "w": w}
        for i in range(N_CORES)
    ]
    res = run_bass_kernel_spmd(
        nc, in_maps, core_ids=list(range(N_CORES)), trace=TRACE
    )
    _cache["last_results"] = res
    outs = []
    for r in res.results:
        pck = np.asarray(r["pck"]).reshape(4, NG, P, F)
        # row 16j+c of group g = packed spikes for t=8g+j, partitions 8c+b
        blk = pck.reshape(4, NG, TG, 16, F)
        bits = np.unpackbits(blk[:, :, :, :, None, :], axis=4, bitorder="little")
        # bits: [tile, g, j, c, b, F] -> [g, j, c, b, tile, F] -> [T, P, Q]
        spk = bits.transpose(1, 2, 3, 4, 0, 5).reshape(T, P, Q)
        outs.append(spk)
    out = np.stack(outs, axis=1).astype(np.float32).reshape(T, NPC * N_CORES)
    return out.reshape(T, 64, 128, 32, 32)


# revision 61
# speedup vs baseline: 1.0023x; 1.0023x over previous
"""LIF spike scan kernel for Trainium2 (8 NeuronCores, data-parallel).

Reference computation (per element, scanned over t):
    mem = mem * 0.2 * (1 - spk) + x[t]
    spk = (mem > 0.5)

Carry formulation with v = post-reset membrane (v = mem * (mem <= 0.5)):
    m = v * 0.2 + x[t]    -> DVE scalar_tensor_tensor, three ops aligned with
                             the v-producer ranges below
    b = (m < 0.5)         -> ACT Sign(0.5-m) -> saturating u8, exact {0,1};
                             three ops split at A1/A so each consumer waits
                             only its own range
    reset, columns [0, A):  v = b * m on Pool tensor_tensor mult (u8 x f32,
                             exact), two ops split at A1
    reset, columns [A, F):  v = (m <= 0.5) * m, DVE fused
                             scalar_tensor_tensor is_le+mult

Output is fully bit-packed on the PE with the u8 mask BITCAST to fp8e4:
byte 0x01 reads as the e4m3 subnormal 2^-9 (a power of two), so with weight
block j (j = t mod 8) holding W[8c+k, 16j+c] = 2^k in fp8e4, PSUM row 16j+c
accumulates 2^-9 * sum_k 2^k * b[8c+k] exactly over 8 timesteps -- all 128
rows useful. One ACT copy PSUM->u8 with scale=-512, bias=255 per group
recovers the true packed spikes (255 - sum 2^k (1-s) = sum 2^k s) exactly
(powers of two throughout), then one [128, 2048] u8 DMA per 8 timesteps.
Output traffic: 2.1 MB/core vs 67.1 MB input (baseline wrote 10.5 MB).
The bitcast removes the bf16 mask copy and the DVE is_le op entirely: one
sign per column serves the Pool reset AND the PE pack.

Weights are shipped from the host as raw fp8e4m3 bit patterns in a uint8
tensor (2^k = (k+7)<<3) and bitcast on device -- 128 KB instead of 512 KB
on the critical DMA stream and no conversion op.

Engine busy per tile-iteration (F=2048 cols, 64 iterations/core):
DMA ~3010 (in 2913 + out/weights), Pool ~2980, DVE ~2980, ACT ~2500,
PE ~850 -> DMA-bound at the 360 GB/s HBM input stream. Pool min/max and
gpsimd stt do not lower on trn2 HW (ISA check), hence the mult-based reset.
Group flips are emitted after both tiles' mask ops, the second pass's
t=0..2 fronts are software-pipelined under the first pass's tail (their
matmuls wait on the PSUM flips at the real boundary), the final timestep
splits its m-update three ways, and the final pass's first-tile flip runs
on the drain-idle DVE (tensor_scalar mult/add from PSUM) so the last
tile's sign chain never queues behind a 1.9us ACT flip.

Only m == 0.5 exactly deviates from the reference (spike recorded as 1 and
v reset to 0 instead of holding; measure-zero under random normal inputs --
4 of 134M elements on the seed-0 input). Everything else is bit-exact fp32.

Sharding: x is [T=16, B=64, C=128, H=32, W=32]; each core takes a contiguous
1/8 of the flattened B*C*H*W axis viewed as [T, 128, 8192]. Two q-tile
chains run concurrently per pass (PSUM holds two [128, 2048] f32 group
accumulators); two passes cover the four q-tiles.
"""

import numpy as np

T = 16
SPATIAL = 64 * 128 * 32 * 32  # 8388608
N_CORES = 8
NPC = SPATIAL // N_CORES      # 1048576 elements per core per timestep
P = 128                       # SBUF partitions
Q = NPC // P                  # 8192 free-dim columns per core
F = 2048                      # free-dim tile size
A = 1408                      # ACT-mask + Pool-mult columns; rest DVE vdn
A1 = 704                      # Pool mult / sign / stt split inside [0, A)
TG = 8                        # timesteps per packed PSUM group
NG = T // TG                  # groups per tile
DECAY = 0.2
THRESH = 0.5

_cache = {}

# Set by test harness to request an NTFF trace / HW timing.
TRACE = False


def _pack_weights() -> np.ndarray:
    """[128, 1024] uint8 holding fp8e4m3 bit patterns, 8 blocks of
    [128, 128]: block j has W[8c+b, 128j + 16j + c] = 2^b for c in [0,16),
    b in [0,8); else 0. fp8e4m3 2^b = (b+7)<<3. Block j lands timestep j's
    packed bytes at PSUM rows 16j..16j+15."""
    w = np.zeros((P, TG * P), dtype=np.uint8)
    for j in range(TG):
        for c in range(16):
            for b in range(8):
                w[8 * c + b, P * j + 16 * j + c] = (b + 7) << 3
    return w


def _build():
    from contextlib import ExitStack

    import concourse.bacc as bacc
    import concourse.tile as tile
    from concourse import mybir

    f32 = mybir.dt.float32
    fp8 = mybir.dt.float8e4
    u8 = mybir.dt.uint8
    Alu = mybir.AluOpType
    Act = mybir.ActivationFunctionType

    nc = bacc.Bacc("TRN2", target_bir_lowering=False, debug=False)
    x_d = nc.dram_tensor("x", [T, P, Q], f32, kind="ExternalInput").ap()
    w_d = nc.dram_tensor("w", [P, TG * P], u8, kind="ExternalInput").ap()
    # Packed planes: [q-tile, group, 128, F]; row 16j+c of group g holds
    # sum_b 2^b * spk[8g+j, 8c+b, col].
    p_d = nc.dram_tensor("pck", [4, NG, P, F], u8, kind="ExternalOutput").ap()

    # Const APs for activation biases (written pre-tile-region; only ACT
    # reads them, so a Pool<->ACT pairwise barrier suffices and SP's first
    # x DMA is not held back by an all-engine barrier).
    for name, val in (("thr", THRESH), ("zero", 0.0), ("flip", 255.0)):
        ap = nc.alloc_sbuf_tensor(f"const-f32-{name}", [128, 1], f32)
        nc.gpsimd.memset(ap.ap(), val)
        nc.const_aps.aps[(f32, val)] = ap.ap()
    nc.multi_engine_barrier(
        [mybir.EngineType.Pool, mybir.EngineType.Activation]
    )

    with tile.TileContext(nc) as tc, ExitStack() as ctx:
        wpool = ctx.enter_context(tc.tile_pool(name="wgt", bufs=1))
        xpool = ctx.enter_context(tc.tile_pool(name="xin", bufs=11))
        vpool = ctx.enter_context(tc.tile_pool(name="vst", bufs=10))
        bpool = ctx.enter_context(tc.tile_pool(name="bms", bufs=9))
        opool = ctx.enter_context(tc.tile_pool(name="out", bufs=4))
        ppool = ctx.enter_context(tc.tile_pool(name="acc", bufs=2, space="PSUM"))

        wu8 = wpool.tile([P, TG * P], u8)
        wb = wu8.bitcast(fp8)

        pend = []  # deferred flip+store: (ti, group, psum tile)

        def flush(n=None, tail=False):
            todo = pend[:n] if n else pend[:]
            del pend[: len(todo)]
            for i, (ti, g, pacc) in enumerate(todo):
                o = opool.tile([P, F], u8, name="po")
                if tail and i == 0:
                    # final pass, first tile: flip on the (idle-at-drain) DVE
                    # so the last tile's sign ops never queue behind a 1.9us
                    # ACT flip
                    nc.vector.tensor_scalar(
                        o[:], pacc[:], -512.0, 255.0,
                        op0=Alu.mult, op1=Alu.add,
                    )
                else:
                    nc.scalar.activation(
                        o[:], pacc[:], Act.Copy, scale=-512.0, bias=255.0
                    )
                nc.scalar.dma_start(p_d[ti, g, :, :], o[:])

        first = True
        state = {}

        def load(ti, t):
            q0 = ti * F
            xt = xpool.tile([P, F], f32, name="xt")
            nc.sync.dma_start(xt[:], x_d[t, :, q0 : q0 + F])
            return xt

        def front_tail(ti, t, vprev):
            """Final tile of the run: chunked load so mask compute pipelines
            under the transfer -- only ~1.4us of work remains after the last
            512-col chunk lands. t=T-1 needs only the pack mask (no reset);
            Sign on the D-range too only moves the measure-zero m==0.5 case."""
            q0 = ti * F
            xt = xpool.tile([P, F], f32, name="xt")
            b = bpool.tile([P, F], u8, name="b")
            for c in range(F // 512):
                ch = slice(512 * c, 512 * (c + 1))
                nc.sync.dma_start(xt[:, ch], x_d[t, :, q0 + 512 * c : q0 + 512 * (c + 1)])
                if vprev is not None:
                    nc.vector.scalar_tensor_tensor(
                        xt[:, ch], vprev[:, ch], DECAY, xt[:, ch],
                        op0=Alu.mult, op1=Alu.add,
                    )
                nc.scalar.activation(
                    b[:, ch], xt[:, ch], Act.Sign, scale=-1.0, bias=THRESH
                )
            return b, None

        def front(ti, t, vprev, xt=None):
            """x load + m-update + masks + reset for (ti, t); mats deferred."""
            nonlocal first
            if xt is None:
                xt = load(ti, t)
            if first:
                # weight bytes (fp8e4 bit patterns) queued behind the first
                # x tile so x[0] isn't delayed
                nc.sync.dma_start(wu8[:], w_d)
                first = False
            m = xt
            b = bpool.tile([P, F], u8, name="b")
            vn = vpool.tile([P, F], f32, name="vn") if t < T - 1 else None
            # m-update in three ops aligned with the apply ranges so each
            # only waits its own v producer
            if vprev is not None:
                # 3-way split at the last step shortens the drain chain
                # (sign3 waits only the short [A,F) piece)
                rs = ((0, A1), (A1, A), (A, F)) if t == T - 1 else ((0, A1), (A1, F))
                for r0, r1 in rs:
                    nc.vector.scalar_tensor_tensor(
                        m[:, r0:r1], vprev[:, r0:r1], DECAY,
                        m[:, r0:r1], op0=Alu.mult, op1=Alu.add,
                    )
            # anti-spike mask b = (m < 0.5): ACT Sign(0.5-m) -> saturating u8,
            # exact {0,1}; split at A1/A so each consumer waits only its range
            nc.scalar.activation(
                b[:, 0:A1], m[:, 0:A1], Act.Sign, scale=-1.0, bias=THRESH
            )
            nc.scalar.activation(
                b[:, A1:A], m[:, A1:A], Act.Sign, scale=-1.0, bias=THRESH
            )
            nc.scalar.activation(
                b[:, A:F], m[:, A:F], Act.Sign, scale=-1.0, bias=THRESH
            )
            if vn is not None:
                # hard reset v = mask * m. The u8 b feeds Pool directly
                # (exact {0,1} multiply).
                nc.gpsimd.tensor_tensor(
                    vn[:, 0:A1], b[:, 0:A1], m[:, 0:A1], op=Alu.mult
                )
                nc.gpsimd.tensor_tensor(
                    vn[:, A1:A], b[:, A1:A], m[:, A1:A], op=Alu.mult
                )
                # [A,F): fused compare+multiply on DVE
                nc.vector.scalar_tensor_tensor(
                    vn[:, A:F], m[:, A:F], THRESH, m[:, A:F],
                    op0=Alu.is_le, op1=Alu.mult,
                )
            return b, vn

        def mats(ti, t, b, acc):
            """pack matmuls for (ti, t) into acc. The u8 mask is bitcast to
            fp8e4 (0x01 == 2^-9 exactly), so PSUM accumulates
            2^-9 * sum_k 2^k * b -- recovered exactly by the flip copy's
            scale=-512."""
            j = t % TG
            for c in range(F // 512):
                ch = slice(512 * c, 512 * (c + 1))
                nc.tensor.matmul(
                    acc[:, ch], wb[:, P * j : P * (j + 1)],
                    b[:, ch].bitcast(mybir.dt.float8e4),
                    start=(j == 0), stop=(j == TG - 1),
                    skip_group_check=True,
                )
            return acc

        for pair in range(2):
            tiles = (2 * pair, 2 * pair + 1)
            v = {ti: None for ti in tiles}
            acc = {ti: None for ti in tiles}
            for t in range(T):
                j = t % TG
                am = {}
                for ti in tiles:
                    if (pair, ti, t) in state:
                        a, vn = state.pop((pair, ti, t))
                    elif pair == 1 and ti == tiles[1] and t == T - 1:
                        a, vn = front_tail(ti, t, v[ti])
                    else:
                        a, vn = front(ti, t, v[ti])
                    v[ti] = vn
                    am[ti] = a
                if j == 0:
                    # flip+store the previous groups after BOTH tiles' mask
                    # ops (no ACT head-of-line burst at the boundary), then
                    # reallocate the PSUM accumulators (matmuls below wait
                    # on the flips via the pool slots)
                    for ti in tiles:
                        if acc[ti] is not None:
                            pend.append((ti, t // TG - 1, acc[ti]))
                    flush()
                    for ti in tiles:
                        acc[ti] = ppool.tile([P, F], f32, name="acc")
                for ti in tiles:
                    mats(ti, t, am[ti], acc[ti])
                if pair == 0 and t >= T - 3:
                    # software-pipeline the pass boundary: the next pass's
                    # first fronts (t=0 has no recurrence inputs; t=1,2 chain
                    # off the prefetched resets) run under this pass's tail;
                    # only their matmuls wait (on the PSUM flips) at the real
                    # boundary
                    nt = t - (T - 3)
                    for nti in (tiles[0] + 2, tiles[1] + 2):
                        vprev = state[(1, nti, nt - 1)][1] if nt else None
                        state[(1, nti, nt)] = front(nti, nt, vprev)
            for ti in tiles:
                pend.append((ti, (T - 1) // TG, acc[ti]))
            flush(tail=(pair == 1))
    nc.compile()
    return nc


def kernel(x: np.ndarray) -> np.ndarray:
    from concourse.bass_utils import run_bass_kernel_spmd

    if "nc" not in _cache:
        _cache["nc"] = _build()
    nc = _cache["nc"]

    x = np.ascontiguousarray(x, dtype=np.float32).reshape(T, N_CORES, NPC)
    w = _pack_weights()
    in_maps = [
        {"x": np.ascontiguousarray(x[:, i]).reshape(T, P, Q), "w": w}
        for i in range(N_CORES)
    ]
    res = run_bass_kernel_spmd(
        nc, in_maps, core_ids=list(range(N_CORES)), trace=TRACE
    )
    _cache["last_results"] = res
    outs = []
    for r in res.results:
        pck = np.asarray(r["pck"]).reshape(4, NG, P, F)
        # row 16j+c of group g = packed spikes for t=8g+j, partitions 8c+b
        blk = pck.reshape(4, NG, TG, 16, F)
        bits = np.unpackbits(blk[:, :, :, :, None, :], axis=4, bitorder="little")
        # bits: [tile, g, j, c, b, F] -> [g, j, c, b, tile, F] -> [T, P, Q]
        spk = bits.transpose(1, 2, 3, 4, 0, 5).reshape(T, P, Q)
        outs.append(spk)
    out = np.stack(outs, axis=1).astype(np.float32).reshape(T, NPC * N_CORES)
    return out.reshape(T, 64, 128, 32, 32)
